# revision 1
# baseline (speedup 1.0000x reference)
"""DinoV2 attention (B=8, S=1370, D=1024, H=16, Dh=64) on 8 trn2 NeuronCores.

Sharding: data parallel over batch — core b computes batch element b end to
end; weights are replicated; no collectives.

Per-core layout strategy (everything feature-major so no device transposes):
  xT   = x[b].T                      [D, S]   (host transposes)
  QT   = Wq.T @ xT  (+bq)            [D, S]   lhsT=Wq tiles (natural layout)
  KT   = Wk.T @ xT  (+bk)            [D, S]
  V    = xT.T @ Wv  (+bv)            [S, D]   lhsT=xT tiles
  ST_h = K_h @ Q_h.T                 [S, S]   lhsT=KT tile, rhs=QT  (keys on
                                              partitions, queries on free dim;
                                              two heads run concurrently in
                                              PE row-groups 0-63 / 64-127)
  E_h  = exp(ST_h / 8)               ScalarE, direct from PSUM
  OT_h = [V_h | 1].T @ E_h           [65, S]  row 64 = softmax denominator Z
  OT   = OT_h / Z  (per head)        [D, S]
  out  = OT.T @ Wo (+bo)             [S, D]   lhsT=OT tiles
All matmul operands are float32r (full PE rate at free dim >= 256); producers
write float32r directly so the BIR verifier sees properly rounded inputs.
"""

import numpy as np
from contextlib import ExitStack

import concourse.bass as bass
import concourse.mybir as mybir
import concourse.tile as tile
from concourse.bass_utils import run_bass_kernel_spmd

B = 8
S = 1370
D = 1024
H = 16
DH = 64
P = 128
KT = D // P          # 8 contraction tiles over D
NPAIR = H // 2       # 8 head pairs
NST = (S + P - 1) // P   # 11 s-tiles (last one is 90 rows)
FP = mybir.dt.float32
FPR = mybir.dt.float32r
AF = mybir.ActivationFunctionType

ST_SIZES = [min(P, S - i * P) for i in range(NST)]
Q_CHUNKS = [(0, 512), (512, 512), (1024, S - 1024)]   # free-dim chunks of S
SCALE = 1.0 / np.sqrt(DH)


def _legalize_syncs(nc):
    """Move excess sem waits onto injected NoOps.

    This walrus build encodes at most one wait (plus one update) per TPB
    instruction; Tile emits several. Engines execute their streams in
    order and the Tile schedule is a topological order of the dependency
    DAG, so hoisting waits onto preceding same-engine NoOps preserves
    progress (anything scheduled earlier can still complete) and
    correctness (the instruction still starts only after all its waits).
    """
    nid = 0
    for f in nc.m.functions:
        for blk in f.blocks:
            out = []
            for inst in blk.instructions:
                si = inst.sync_info
                if si is not None:
                    waits = list(si.on_wait)
                    ups = list(si.on_update)
                    if len(waits) > 1:
                        for w in waits[:-1]:
                            nop = mybir.InstNoOp(
                                name=f"I-syncfix-{nid}",
                                engine=inst.engine, ins=[], outs=[],
                                sync_info=mybir.SyncInfo(on_wait=[w],
                                                         on_update=[]))
                            nid += 1
                            nc.register_instruction(nop)
                            out.append(nop)
                        inst.sync_info = mybir.SyncInfo(on_wait=waits[-1:],
                                                        on_update=ups)
                out.append(inst)
            blk.instructions = out


def build_nc(repeat=1):
    nc = bass.Bass()
    xT = nc.declare_dram_parameter("xT", [D, S], FPR, isOutput=False)
    Wq = nc.declare_dram_parameter("Wq", [D, D], FPR, isOutput=False)
    Wk = nc.declare_dram_parameter("Wk", [D, D], FPR, isOutput=False)
    Wv = nc.declare_dram_parameter("Wv", [D, D], FPR, isOutput=False)
    Wo = nc.declare_dram_parameter("Wo", [D, D], FPR, isOutput=False)
    bq = nc.declare_dram_parameter("bq", [D], FP, isOutput=False)
    bk = nc.declare_dram_parameter("bk", [D], FP, isOutput=False)
    bv = nc.declare_dram_parameter("bv", [D], FP, isOutput=False)
    bo = nc.declare_dram_parameter("bo", [D], FP, isOutput=False)
    out = nc.declare_dram_parameter("out", [S, D], FP, isOutput=True)
    # bounce buffer for replicating softmax denominators across partitions
    # (SBUF APs can't partition-broadcast, DRAM APs can)
    zdram = nc.dram_tensor("zbounce", [NPAIR, 3, 1024], FP)

    def bcast128(handle):
        # [D] dram vector replicated across 128 partitions
        a = handle[:]
        return bass.AP(tensor=a.tensor, offset=a.offset,
                       ap=[[0, P], *a.ap])

    with ExitStack() as ctx:
        tc = ctx.enter_context(tile.TileContext(nc))
        const = ctx.enter_context(tc.tile_pool(name="const", bufs=1))
        persist = ctx.enter_context(tc.tile_pool(name="persist", bufs=1))
        psum = ctx.enter_context(tc.tile_pool(name="psum", bufs=1, space="PSUM"))

        # biases: per-partition layout for QT/KT (feature on partitions),
        # partition-broadcast layout for V / out (feature on free dim)
        bq_sb = const.tile([P, KT], FP)
        nc.sync.dma_start(out=bq_sb, in_=bq[:].rearrange("(kt p) -> p kt", p=P))
        bk_sb = const.tile([P, KT], FP)
        nc.sync.dma_start(out=bk_sb, in_=bk[:].rearrange("(kt p) -> p kt", p=P))

        # persistent: V (with fused ones column per head) and normalized OT
        v_sb = persist.tile([P, NST, H, DH + 1], FPR)
        ot_sb = persist.tile([P, KT, S], FPR)
        for st in range(NST):
            for h in range(H):
                nc.vector.memset(v_sb[:, st, h, DH:DH + 1].bitcast(FP), 1.0)

        for _rep in range(repeat):
            with tc.tile_pool(name="xp", bufs=1) as xp:
                xT_sb = xp.tile([P, KT, S], FPR)
                xT_r = xT[:].rearrange("(kt p) s -> p kt s", p=P)

                wp_cm = tc.tile_pool(name="wp", bufs=2)
                wp = wp_cm.__enter__()
                wq_pre = wp.tile([P, KT, P], FPR, tag="wq_t")
                wk_pre = wp.tile([P, KT, P], FPR, tag="wk_t")

                # ---- Phase A: V projection for all heads, in two 512-col halves
                with tc.tile_pool(name="wvp", bufs=2) as wvp:
                    wv_full_r = Wv[:].rearrange("(kt p) c -> p kt c", p=P)
                    # DMA emission order sets arrival order: the first wv
                    # chunk goes out ahead of xT so the very first V-proj
                    # matmul is unblocked ~3us in, instead of waiting for
                    # the whole 5.6MB xT stream to drain first
                    wv_tiles = []
                    for half in range(2):
                        wv_sb = wvp.tile([P, KT, 512], FPR)
                        wv_tiles.append(wv_sb)
                    nc.sync.dma_start(out=wv_tiles[0][:, 0:4, :],
                                      in_=wv_full_r[:, 0:4, 0:512])
                    for kt in range(KT):
                        nc.sync.dma_start(out=xT_sb[:, kt, :],
                                          in_=xT_r[:, kt, :])
                        if kt == 3:
                            nc.sync.dma_start(out=wv_tiles[0][:, 4:8, :],
                                              in_=wv_full_r[:, 4:8, 0:512])
                    bv_bc = wvp.tile([P, D], FP, tag="bvbc", bufs=1)
                    nc.sync.dma_start(out=bv_bc, in_=bcast128(bv))
                    for g in range(2):
                        nc.sync.dma_start(
                            out=wv_tiles[1][:, g * 4:(g + 1) * 4, :],
                            in_=wv_full_r[:, g * 4:(g + 1) * 4, 512:1024])
                    # prefetch pair-0 Q/K weights during phase A so phase B
                    # starts without a weight-load bubble
                    nc.sync.dma_start(
                        out=wq_pre,
                        in_=Wq[:].rearrange("(kt p) c -> p kt c", p=P)[:, :, 0:P])
                    nc.sync.dma_start(
                        out=wk_pre,
                        in_=Wk[:].rearrange("(kt p) c -> p kt c", p=P)[:, :, 0:P])
                    for half in range(2):
                        c0 = half * 512
                        wv_sb = wv_tiles[half]
                        h0 = half * 8

                        def v_evict(vps_prev, st_prev):
                            swp = ST_SIZES[st_prev]
                            nc.vector.tensor_add(
                                v_sb[:swp, st_prev, h0:h0 + 8, 0:DH],
                                vps_prev[:swp, :].rearrange("p (h d) -> p h d",
                                                            h=8),
                                bv_bc[:swp, c0:c0 + 512].rearrange(
                                    "p (h d) -> p h d", h=8))

                        pend = None
                        for st in range(NST):
                            sw = ST_SIZES[st]
                            vps = psum.tile([P, 512], FP,
                                            tag="mm",
                                            bufs=8)
                            for kt in range(KT):
                                nc.tensor.matmul(
                                    vps[:sw, :],
                                    lhsT=(xT_sb[:, kt, st * P:st * P + sw]),
                                    rhs=(wv_sb[:, kt, :]),
                                    start=(kt == 0), stop=(kt == KT - 1))
                            if pend is not None:
                                v_evict(*pend)
                            pend = (vps, st)
                        v_evict(*pend)

                # ---- Phase B: per head pair: QT/KT projections + attention
                with tc.tile_pool(name="qkp", bufs=2) as qkp, \
                     tc.tile_pool(name="ep", bufs=2) as ep, \
                     tc.tile_pool(name="zp", bufs=2) as zp, \
                     tc.tile_pool(name="obp", bufs=2) as obp:
                    for hp in range(NPAIR):
                        if hp == 0:
                            wq_t, wk_t = wq_pre, wk_pre
                        else:
                            wq_t = wp.tile([P, KT, P], FPR, tag="wq_t")
                            wk_t = wp.tile([P, KT, P], FPR, tag="wk_t")
                            nc.sync.dma_start(
                                out=wq_t,
                                in_=Wq[:].rearrange(
                                    "(kt p) c -> p kt c",
                                    p=P)[:, :, hp * P:(hp + 1) * P])
                            nc.sync.dma_start(
                                out=wk_t,
                                in_=Wk[:].rearrange(
                                    "(kt p) c -> p kt c",
                                    p=P)[:, :, hp * P:(hp + 1) * P])
                        qt_sb = qkp.tile([P, S], FPR)
                        kt_sb = qkp.tile([P, S], FPR)
                        for (q0, cw) in Q_CHUNKS:
                            qps = psum.tile([P, 512], FP, tag="mm", bufs=8)
                            for kt in range(KT):
                                nc.tensor.matmul(
                                    qps[:, :cw], lhsT=(wq_t[:, kt, :]),
                                    rhs=(xT_sb[:, kt, q0:q0 + cw]),
                                    start=(kt == 0), stop=(kt == KT - 1))
                            nc.vector.tensor_scalar_add(
                                qt_sb[:, q0:q0 + cw], qps[:, :cw],
                                bq_sb[:, hp:hp + 1])
                            kps = psum.tile([P, 512], FP, tag="mm", bufs=8)
                            for kt in range(KT):
                                nc.tensor.matmul(
                                    kps[:, :cw], lhsT=(wk_t[:, kt, :]),
                                    rhs=(xT_sb[:, kt, q0:q0 + cw]),
                                    start=(kt == 0), stop=(kt == KT - 1))
                            nc.vector.tensor_scalar_add(
                                kt_sb[:, q0:q0 + cw], kps[:, :cw],
                                bk_sb[:, hp:hp + 1])

                        # attention for heads (2*hp, 2*hp+1); A in array rows
                        # 0-63, B in rows 64-127, running concurrently
                        for qci, (q0, cw) in enumerate(Q_CHUNKS):
                            oA = psum.tile([DH + 1, 512], FP, tag="mm", bufs=8)
                            oB = psum.tile([DH + 1, 512], FP, tag="mm", bufs=8)
                            def pv(eA, eB, kw, ks):
                                nc.tensor.matmul(
                                    oA[:, :cw], lhsT=(v_sb[0:kw, ks, 2 * hp, :]),
                                    rhs=(eA[:kw, :cw]),
                                    start=(ks == 0), stop=(ks == NST - 1))
                                nc.tensor.matmul(
                                    oB[:, :cw],
                                    lhsT=(v_sb[0:kw, ks, 2 * hp + 1, :]),
                                    rhs=(eB[:kw, :cw]),
                                    start=(ks == 0), stop=(ks == NST - 1))

                            pend = None
                            for ks in range(NST):
                                k0, kw = ks * P, ST_SIZES[ks]
                                sA = psum.tile([P, 512], FP, tag="mm", bufs=8)
                                sB = psum.tile([P, 512], FP, tag="mm", bufs=8)
                                nc.tensor.matmul(
                                    sA[:kw, :cw],
                                    lhsT=(kt_sb[0:DH, k0:k0 + kw]),
                                    rhs=(qt_sb[0:DH, q0:q0 + cw]),
                                    start=True, stop=True, tile_position=(0, 0))
                                nc.tensor.matmul(
                                    sB[:kw, :cw],
                                    lhsT=(kt_sb[DH:P, k0:k0 + kw]),
                                    rhs=(qt_sb[DH:P, q0:q0 + cw]),
                                    start=True, stop=True, tile_position=(64, 0))
                                eA = ep.tile([P, 512], FPR)
                                eB = ep.tile([P, 512], FPR)
                                nc.scalar.activation(eA[:kw, :cw], sA[:kw, :cw],
                                                     AF.Exp, scale=float(SCALE))
                                nc.scalar.activation(eB[:kw, :cw], sB[:kw, :cw],
                                                     AF.Exp, scale=float(SCALE))
                                if pend is not None:
                                    pv(*pend)
                                pend = (eA, eB, kw, ks)
                            pv(*pend)
                            # normalize: row DH of oA/oB is Z; reciprocal stays on
                            # partition 64 (no cross-partition compute), then DMA
                            # replicates it across partitions 0-63
                            zt = zp.tile([P, 1024], FP)
                            if cw < 512:
                                # the bounce DMA reads the whole row; fill
                                # the never-written tail so stale-slot reads
                                # see defined data
                                nc.vector.memset(zt[DH:DH + 1, cw:512], 1.0)
                                nc.vector.memset(zt[DH:DH + 1, 512 + cw:1024],
                                                 1.0)
                            nc.vector.reciprocal(zt[DH:DH + 1, 0:cw],
                                                 oA[DH:DH + 1, :cw])
                            nc.vector.reciprocal(zt[DH:DH + 1, 512:512 + cw],
                                                 oB[DH:DH + 1, :cw])
                            # bounce through DRAM to replicate across partitions
                            nc.sync.dma_start(out=zdram[hp, qci, :],
                                              in_=zt[DH:DH + 1, :])
                            zb = zp.tile([DH, 1024], FP)
                            zsrc = zdram[hp, qci, :]
                            nc.sync.dma_start(
                                out=zb,
                                in_=bass.AP(tensor=zsrc.tensor, offset=zsrc.offset,
                                            ap=[[0, DH], *zsrc.ap]))
                            # head A rows live at partitions 0-63 of ot tile hp
                            nc.vector.tensor_mul(
                                ot_sb[0:DH, hp, q0:q0 + cw],
                                oA[0:DH, :cw], zb[:, 0:cw])
                            # head B rows must land at partitions 64-127; compute
                            # at base 0 then DMA-shift partitions
                            otB = obp.tile([DH, 512], FPR)
                            nc.vector.tensor_mul(otB[:, :cw], oB[0:DH, :cw],
                                                 zb[:, 512:512 + cw])
                            nc.sync.dma_start(
                                out=ot_sb[DH:P, hp, q0:q0 + cw],
                                in_=otB[:, :cw])

                wp_cm.__exit__(None, None, None)

            # ---- Phase C: out = OT.T @ Wo + bo, head-halves A/B run in
            # concurrent PE row-groups accumulating into separate PSUM banks
            with tc.tile_pool(name="wop", bufs=1) as wop, \
                 tc.tile_pool(name="outp", bufs=3) as outp:
                wo_sb = wop.tile([P, KT, D], FPR)
                bo_bc = outp.tile([P, D], FP, tag="bobc", bufs=1)
                nc.sync.dma_start(out=bo_bc, in_=bcast128(bo))
                wo_r = Wo[:].rearrange("(kt p) c -> p kt c", p=P)
                for g in range(2):
                    nc.sync.dma_start(out=wo_sb[:, g * 4:(g + 1) * 4, :],
                                      in_=wo_r[:, g * 4:(g + 1) * 4, :])
                for st in range(NST):
                    sw = ST_SIZES[st]
                    s0 = st * P
                    for (c0, cw2) in [(0, 512), (512, 512)]:
                        psA = psum.tile([P, 512], FP, tag="mm", bufs=8)
                        psB = psum.tile([P, 512], FP, tag="mm", bufs=8)
                        for dt in range(KT):
                            nc.tensor.matmul(
                                psA[:sw, :cw2],
                                lhsT=(ot_sb[0:DH, dt, s0:s0 + sw]),
                                rhs=(wo_sb[0:DH, dt, c0:c0 + cw2]),
                                start=(dt == 0), stop=(dt == KT - 1),
                                tile_position=(0, 0))
                            nc.tensor.matmul(
                                psB[:sw, :cw2],
                                lhsT=(ot_sb[DH:P, dt, s0:s0 + sw]),
                                rhs=(wo_sb[DH:P, dt, c0:c0 + cw2]),
                                start=(dt == 0), stop=(dt == KT - 1),
                                tile_position=(64, 0))
                        o_sb = outp.tile([P, 512], FP)
                        # DVE may read only one PSUM operand per instruction
                        nc.vector.tensor_add(o_sb[:sw, :cw2], psA[:sw, :cw2],
                                             bo_bc[:sw, c0:c0 + cw2])
                        nc.vector.tensor_add(o_sb[:sw, :cw2], o_sb[:sw, :cw2],
                                             psB[:sw, :cw2])
                        nc.sync.dma_start(out=out[s0:s0 + sw, c0:c0 + cw2],
                                          in_=o_sb[:sw, :cw2])
    _legalize_syncs(nc)
    return nc


_NC_CACHE = []


def _get_nc():
    if not _NC_CACHE:
        _NC_CACHE.append(build_nc())
    return _NC_CACHE[0]


def _in_maps(x, Wq, bq, Wk, bk, Wv, bv, Wo, bo):
    f = lambda a: np.ascontiguousarray(np.asarray(a, dtype=np.float32))
    shared = {"Wq": f(Wq), "Wk": f(Wk), "Wv": f(Wv), "Wo": f(Wo),
              "bq": f(bq), "bk": f(bk), "bv": f(bv), "bo": f(bo)}
    x = np.asarray(x, dtype=np.float32)
    return [{"xT": np.ascontiguousarray(x[b].T), **shared} for b in range(B)]


def kernel(x, Wq, bq, Wk, bk, Wv, bv, Wo, bo):
    nc = _get_nc()
    in_maps = _in_maps(x, Wq, bq, Wk, bk, Wv, bv, Wo, bo)
    res = run_bass_kernel_spmd(nc, in_maps, list(range(B)))
    return np.stack([res.results[b]["out"] for b in range(B)], axis=0)



# revision 29
# speedup vs baseline: 1.2543x; 1.2543x over previous
"""DinoV2 attention (B=8, S=1370, D=1024, H=16, Dh=64) on 8 trn2 NeuronCores.

Sharding: data parallel over batch - core b computes batch element b end to
end; weights are replicated; no collectives.

All matmul operands are bf16 (same PE rate as fp32r in the cost model but
half the DMA/SBUF footprint); PSUM accumulation stays fp32.

Structure (per core):
  Phase A: V = x @ Wv + bv (token-major, 512-col chunks) and K^T = Wk^T x
           (feature-major, all keys) back to back, paced by the input DMA
           stream, with all 8 PSUM banks in one rotating ring.
  Main loop, query-chunk-outer (5x256 + 90), head-pair-inner:
    scores S_h = K_h^T Q_h   [keys, qchunk]; two ks tiles x two heads are
    packed into one 2-bank PSUM tile so ONE Act instruction exponentiates
    1024 columns (Act instruction overhead halves).
    O_h = [V_h | 1]^T E_h    [65, qchunk], accumulated over 11 key tiles.
    normalize via reciprocal of row 64 + DRAM-bounce partition broadcast.
    The next iteration's Q-projection passes and the previous query chunk's
    out-projection passes are woven between the attention matmuls so the PE
    never waits on the Act engine's exp stream.
  Out-projection is feature-major (outT = Wo^T @ OT, full 128-deep
  contraction, per-partition bias); the host transposes outT back.
"""

import numpy as np
from contextlib import ExitStack

import concourse.bass as bass
import concourse.mybir as mybir
import concourse.tile as tile
from concourse.bass_utils import run_bass_kernel_spmd

import ml_dtypes
_BF16 = np.dtype(ml_dtypes.bfloat16)

B = 8
S = 1370
D = 1024
H = 16
DH = 64
P = 128
KT = D // P              # 8 contraction tiles over D
NPAIR = H // 2           # 8 head pairs
NST = (S + P - 1) // P   # 11 key tiles (last is 90 rows)
NKP = (NST + 1) // 2     # 6 ks pairs (last is a single)
FP = mybir.dt.float32
BF = mybir.dt.bfloat16
AF = mybir.ActivationFunctionType

ST_SIZES = [min(P, S - i * P) for i in range(NST)]
Q_CHUNKS = [(i * 256, min(256, S - i * 256)) for i in range((S + 255) // 256)]
NQC = len(Q_CHUNKS)      # 6 chunks: 5x256 + 90
K_CHUNKS = [(0, 512), (512, 512), (1024, S - 1024)]
SCALE = 1.0 / np.sqrt(DH)
ONES_BF16_BITS = 0x3F80  # 1.0 in bfloat16


def _legalize_syncs(nc):
    """Move excess sem waits onto injected NoOps.

    This walrus build encodes at most one wait (plus one update) per TPB
    instruction; Tile emits several. Engines execute their streams in
    order and the Tile schedule is a topological order of the dependency
    DAG, so hoisting waits onto preceding same-engine NoOps preserves
    progress (anything scheduled earlier can still complete) and
    correctness (the instruction still starts only after all its waits).
    """
    nid = 0
    for f in nc.m.functions:
        for blk in f.blocks:
            out = []
            for inst in blk.instructions:
                si = inst.sync_info
                if si is not None:
                    waits = list(si.on_wait)
                    ups = list(si.on_update)
                    if len(waits) > 1:
                        for w in waits[:-1]:
                            nop = mybir.InstNoOp(
                                name=f"I-syncfix-{nid}",
                                engine=inst.engine, ins=[], outs=[],
                                sync_info=mybir.SyncInfo(on_wait=[w],
                                                         on_update=[]))
                            nid += 1
                            nc.register_instruction(nop)
                            out.append(nop)
                        inst.sync_info = mybir.SyncInfo(on_wait=waits[-1:],
                                                        on_update=ups)
                out.append(inst)
            blk.instructions = out


def build_nc():
    nc = bass.Bass()
    xT = nc.declare_dram_parameter("xT", [D, S], BF, isOutput=False)
    Wq = nc.declare_dram_parameter("Wq", [D, D], BF, isOutput=False)
    Wk = nc.declare_dram_parameter("Wk", [D, D], BF, isOutput=False)
    Wv = nc.declare_dram_parameter("Wv", [D, D], BF, isOutput=False)
    Wo = nc.declare_dram_parameter("Wo", [D, D], BF, isOutput=False)
    bq = nc.declare_dram_parameter("bq", [D], FP, isOutput=False)
    bk = nc.declare_dram_parameter("bk", [D], FP, isOutput=False)
    bv = nc.declare_dram_parameter("bv", [D], FP, isOutput=False)
    bo = nc.declare_dram_parameter("bo", [D], FP, isOutput=False)
    outT = nc.declare_dram_parameter("outT", [D, S], FP, isOutput=True)
    # bounce buffer for replicating softmax denominators across partitions
    # (SBUF APs can't partition-broadcast, DRAM APs can)
    zdram = nc.dram_tensor("zbounce", [NPAIR, NQC, 2, 256], FP)

    def bcast(handle_slice, nparts):
        a = handle_slice
        return bass.AP(tensor=a.tensor, offset=a.offset,
                       ap=[[0, nparts], *a.ap])

    with ExitStack() as ctx:
        tc = ctx.enter_context(tile.TileContext(nc))
        const = ctx.enter_context(tc.tile_pool(name="const", bufs=1))
        persist = ctx.enter_context(tc.tile_pool(name="persist", bufs=1))
        work = ctx.enter_context(tc.tile_pool(name="work", bufs=1))

        # ---- resident weights / inputs (all bf16) -----------------------
        xT_sb = const.tile([P, KT, S], BF)
        wq_sb = const.tile([P, KT, D], BF)
        wk_sb = const.tile([P, KT, D], BF)
        wv_sb = const.tile([P, KT, D], BF)
        wo_sb = const.tile([P, KT, D], BF)
        # per-partition biases for feature-major projections: feature
        # f = blk*128 + p  ->  [p, blk]
        bq_sb = const.tile([P, KT], FP)
        bk_sb = const.tile([P, KT], FP)
        bo_sb = const.tile([P, KT], FP)
        # V-proj bias is per free-dim column: partition-broadcast via DRAM
        bv_bc = const.tile([P, D], FP)

        xT_r = xT[:].rearrange("(kt p) s -> p kt s", p=P)
        wq_r = Wq[:].rearrange("(kt p) c -> p kt c", p=P)
        wk_r = Wk[:].rearrange("(kt p) c -> p kt c", p=P)
        wv_r = Wv[:].rearrange("(kt p) c -> p kt c", p=P)
        wo_r = Wo[:].rearrange("(kt p) c -> p kt c", p=P)

        # DMA order = arrival order: wv/xT chunk pairs feed phase A first,
        # then wk (phase A's K projection), then the rest.
        nc.sync.dma_start(out=wv_sb[:, 0, :], in_=wv_r[:, 0, :])
        nc.sync.dma_start(out=xT_sb[:, 0, :], in_=xT_r[:, 0, :])
        nc.sync.dma_start(out=bv_bc, in_=bcast(bv[:], P))
        for kt in range(1, KT):
            nc.sync.dma_start(out=wv_sb[:, kt, :], in_=wv_r[:, kt, :])
            nc.sync.dma_start(out=xT_sb[:, kt, :], in_=xT_r[:, kt, :])
        nc.sync.dma_start(out=bk_sb, in_=bk[:].rearrange("(kt p) -> p kt", p=P))
        nc.sync.dma_start(out=bq_sb, in_=bq[:].rearrange("(kt p) -> p kt", p=P))
        nc.sync.dma_start(out=bo_sb, in_=bo[:].rearrange("(kt p) -> p kt", p=P))
        for kt in range(KT):
            nc.sync.dma_start(out=wk_sb[:, kt, :], in_=wk_r[:, kt, :])
        for kt in range(KT):
            nc.sync.dma_start(out=wq_sb[:, kt, :], in_=wq_r[:, kt, :])
        for kt in range(KT):
            nc.sync.dma_start(out=wo_sb[:, kt, :], in_=wo_r[:, kt, :])

        # ---- persistent activations ------------------------------------
        # V with fused ones column: [keys, st, head, 65]
        v_sb = persist.tile([P, NST, H, DH + 1], BF)
        # K^T feature-major for all head pairs: [feat, hp, key]
        kt_all = persist.tile([P, NPAIR, S], BF)
        # normalized attention output, feature-major: [feat, hp, query]
        ot_sb = persist.tile([P, KT, S], BF)
        nc.vector.memset(
            v_sb[:, :, :, DH:DH + 1].bitcast(mybir.dt.uint16), ONES_BF16_BITS)

        # ---- Phase A: V projection then K^T projection ------------------
        # One 8-bank PSUM ring; V regions are paced by the wv/xT DMA
        # stream, K regions run at full speed once wk has landed.
        with tc.tile_pool(name="psA", bufs=1, space="PSUM") as psA:
            for st in range(NST):
                sw = ST_SIZES[st]
                for half in range(2):
                    c0 = half * 512
                    vps = psA.tile([P, 512], FP, tag="vp", bufs=8)
                    for kt in range(KT):
                        nc.tensor.matmul(
                            vps[:sw, :],
                            lhsT=xT_sb[:, kt, st * P:st * P + sw],
                            rhs=wv_sb[:, kt, c0:c0 + 512],
                            start=(kt == 0), stop=(kt == KT - 1))
                    # evict with bias add; 512 cols = 8 heads x 64 dims
                    nc.vector.tensor_add(
                        v_sb[:sw, st, half * 8:half * 8 + 8, 0:DH],
                        vps[:sw, :].rearrange("p (h d) -> p h d", h=8),
                        bv_bc[:sw, c0:c0 + 512].rearrange(
                            "p (h d) -> p h d", h=8))
            for hp in range(NPAIR):
                for (c0, cwk) in K_CHUNKS:
                    kps = psA.tile([P, 512], FP, tag="vp", bufs=8)
                    for kt in range(KT):
                        nc.tensor.matmul(
                            kps[:, :cwk],
                            lhsT=wk_sb[:, kt, hp * P:(hp + 1) * P],
                            rhs=xT_sb[:, kt, c0:c0 + cwk],
                            start=(kt == 0), stop=(kt == KT - 1))
                    nc.vector.tensor_scalar_add(
                        kt_all[:, hp, c0:c0 + cwk], kps[:, :cwk],
                        bk_sb[:, hp:hp + 1])

        # ---- main loop: query-chunk-outer, head-pair-inner --------------
        psum = ctx.enter_context(tc.tile_pool(name="psum", bufs=1,
                                              space="PSUM"))
        qt_tiles = {}
        deferred = [None]

        def emit_qproj(hp, qc):
            """Q projection for (hp, query chunk qc)."""
            q0, cw = Q_CHUNKS[qc]
            q_ps = psum.tile([P, 512], FP, tag="qk", bufs=1)
            ops = []
            for kt in range(KT):
                def mm(kt=kt):
                    nc.tensor.matmul(
                        q_ps[:, :cw],
                        lhsT=wq_sb[:, kt, hp * P:(hp + 1) * P],
                        rhs=xT_sb[:, kt, q0:q0 + cw],
                        start=(kt == 0), stop=(kt == KT - 1))
                ops.append(mm)

            def evict():
                qt = work.tile([P, 256], BF, tag="qt", bufs=2)
                nc.vector.tensor_scalar_add(qt[:, :cw], q_ps[:, :cw],
                                            bq_sb[:, hp:hp + 1])
                qt_tiles[(qc, hp)] = qt
            ops.append(evict)
            return ops

        def emit_outproj(qc, pool_tag="op"):
            """Out-projection passes for query chunk qc (all heads done)."""
            q0, cw = Q_CHUNKS[qc]
            ops = []
            op_bank = None
            if pool_tag == "op":
                op_bank = psum.tile([P, 2, 256], FP, tag="op", bufs=1,
                                    name="op_bank")
            for ct in range(KT):
                if pool_tag == "op":
                    # alternate half-bank regions: double-buffering inside
                    # one PSUM bank
                    region = op_bank[:, ct % 2, :cw]
                else:
                    region = None
                for dt in range(KT):
                    def mm(ct=ct, dt=dt, region=region):
                        nc.tensor.matmul(
                            region,
                            lhsT=wo_sb[:, dt, ct * P:(ct + 1) * P],
                            rhs=ot_sb[:, dt, q0:q0 + cw],
                            start=(dt == 0), stop=(dt == KT - 1))
                    ops.append(mm)

                def evict(ct=ct, region=region):
                    o_out = work.tile([P, 256], FP, tag="oout", bufs=2)
                    nc.vector.tensor_scalar_add(o_out[:, :cw], region,
                                                bo_sb[:, ct:ct + 1])
                    nc.sync.dma_start(
                        out=outT[ct * P:(ct + 1) * P, q0:q0 + cw],
                        in_=o_out[:, :cw])
                ops.append(evict)
            return ops

        def emit_outproj_final(qc):
            """Final out-projection in two batches of 4 ct tiles, each ct's
            accumulation group in its own PSUM bank (concurrent groups must
            not share a bank). dt-major order for dt<7 lets those passes
            overlap the last head pair's normalize chain, whose result only
            the dt=7 passes need."""
            q0, cw = Q_CHUNKS[qc]
            assert cw <= 256
            for batch in range(2):
                t0 = psum.tile([P, 4, 256], FP, tag="s", bufs=2, name="fin0")
                t1 = psum.tile([P, 4, 256], FP, tag="s", bufs=2, name="fin1")
                cts = range(batch * 4, batch * 4 + 4)
                region = {ct: (t0 if i < 2 else t1)[:, 2 * (i % 2), :cw]
                          for i, ct in enumerate(cts)}
                for dt in range(KT):
                    for ct in cts:
                        nc.tensor.matmul(
                            region[ct],
                            lhsT=wo_sb[:, dt, ct * P:(ct + 1) * P],
                            rhs=ot_sb[:, dt, q0:q0 + cw],
                            start=(dt == 0), stop=(dt == KT - 1))
                for ct in cts:
                    o_out = work.tile([P, 256], FP, tag="oout", bufs=2)
                    nc.vector.tensor_scalar_add(o_out[:, :cw], region[ct],
                                                bo_sb[:, ct:ct + 1])
                    nc.sync.dma_start(
                        out=outT[ct * P:(ct + 1) * P, q0:q0 + cw],
                        in_=o_out[:, :cw])

        def attention(qc, hp, fill):
            """Attention for (qc, hp); pops fill closures between matmuls."""
            q0, cw = Q_CHUNKS[qc]
            qt = qt_tiles.pop((qc, hp))
            # heads in SEPARATE banks: two accumulation groups may not share
            # a PSUM bank's zero region
            o_ps = psum.tile([P, 2, 512], FP, tag="o", bufs=1)

            def scores_exp(kp):
                ks_list = ([2 * kp, 2 * kp + 1] if kp < NKP - 1
                           else [NST - 1])
                # head-major region order: head A tiles live in bank 0,
                # head B tiles in bank 1 - consecutive matmuls with
                # different tile_position row groups must not target the
                # same PSUM bank (device lockup otherwise)
                s_ps = psum.tile([P, 2, 2, 256], FP, tag="s", bufs=2)
                for i, ks in enumerate(ks_list):
                    k0, kw = ks * P, ST_SIZES[ks]
                    nc.tensor.matmul(
                        s_ps[:kw, 0, i, :cw],
                        lhsT=kt_all[0:DH, hp, k0:k0 + kw],
                        rhs=qt[0:DH, :cw], start=True, stop=True)
                    nc.tensor.matmul(
                        s_ps[:kw, 1, i, :cw],
                        lhsT=kt_all[DH:P, hp, k0:k0 + kw],
                        rhs=qt[DH:P, :cw], start=True, stop=True)
                kw0 = ST_SIZES[ks_list[0]]
                e = work.tile([P, 2, 2, 256], BF, tag="e", bufs=5)
                n = len(ks_list)
                nc.scalar.activation(
                    e[:kw0, :, 0:n, :cw],
                    s_ps[:kw0, :, 0:n, :cw],
                    AF.Exp, scale=float(SCALE))
                return (e, ks_list)

            def pv(pend):
                e, ks_list = pend
                for i, ks in enumerate(ks_list):
                    kw = ST_SIZES[ks]
                    for h in range(2):
                        nc.tensor.matmul(
                            o_ps[0:DH + 1, h, :cw],
                            lhsT=v_sb[0:kw, ks, 2 * hp + h, :],
                            rhs=e[:kw, h, i, :cw],
                            start=(ks == 0), stop=(ks == NST - 1))

            # depth-2 software pipeline: pv lags scores by two ks pairs so
            # each exp has ~2 pair-times of latency slack
            pend = [scores_exp(0)]
            # previous iteration's deferred PVs + normalize: running them
            # here gives their exps a full extra pair of latency slack
            if deferred[0] is not None:
                deferred[0]()
                deferred[0] = None
            pend.append(scores_exp(1))
            for kp in range(2, NKP):
                pend.append(scores_exp(kp))
                for _ in range(4):
                    if fill:
                        fill.pop(0)()
                pv(pend.pop(0))
            while fill:
                fill.pop(0)()

            def tail(pend=pend, o_ps=o_ps, cw=cw, q0=q0, hp=hp, qc=qc):
                for p in pend:
                    pv(p)
                # copy O out of PSUM immediately so the o banks free for the
                # next iteration (o is single-buffered)
                ocp = work.tile([DH + 1, 2, 256], FP, tag="ocp", bufs=2)
                nc.vector.tensor_copy(ocp[:, :, :cw],
                                      o_ps[0:DH + 1, :, :cw])
                # normalize: row DH holds Z; reciprocal stays on partition
                # 64, then a DRAM bounce replicates it across partitions 0-63
                zt = work.tile([1, 2, 256], FP, tag="zt", bufs=2)
                nc.vector.reciprocal(zt[0:1, :, :cw],
                                     ocp[DH:DH + 1, :, :cw])
                nc.sync.dma_start(out=zdram[hp, qc, :, :cw],
                                  in_=zt[0:1, :, :cw])
                zb = work.tile([DH, 2, 256], FP, tag="zb", bufs=2)
                zsrc = zdram[hp, qc, :, :cw]
                nc.sync.dma_start(out=zb[:, :, :cw], in_=bcast(zsrc, DH))
                # head A lands on partitions 0-63 directly
                nc.vector.tensor_mul(ot_sb[0:DH, hp, q0:q0 + cw],
                                     ocp[0:DH, 0, :cw], zb[:, 0, :cw])
                # head B must land on partitions 64-127: compute at base 0,
                # then DMA-shift partitions
                otb = work.tile([DH, 256], BF, tag="otb", bufs=2)
                nc.vector.tensor_mul(otb[:, :cw], ocp[0:DH, 1, :cw],
                                     zb[:, 1, :cw])
                nc.sync.dma_start(out=ot_sb[DH:P, hp, q0:q0 + cw],
                                  in_=otb[:, :cw])
            deferred[0] = tail

        # lead block: Q projection for the very first iteration
        for op in emit_qproj(0, 0):
            op()

        op_pending = []
        n_it = 0
        for qc in range(NQC):
            for hp in range(NPAIR):
                if LIMIT is not None and n_it >= LIMIT[0]:
                    break
                n_it += 1
                fill = []
                nhp, nqc = hp + 1, qc
                if nhp == NPAIR:
                    nhp, nqc = 0, qc + 1
                if nqc < NQC:
                    fill.extend(emit_qproj(nhp, nqc))
                if qc > 0:
                    if hp == 0:
                        op_pending = emit_outproj(qc - 1)
                    take = 9 if hp < NPAIR - 1 else len(op_pending)
                    fill.extend(op_pending[:take])
                    del op_pending[:take]
                attention(qc, hp, fill)
        if deferred[0] is not None:
            deferred[0]()
            deferred[0] = None
        # final out-projection for the last query chunk
        if LIMIT is None or LIMIT[1]:
            emit_outproj_final(NQC - 1)

    _legalize_syncs(nc)
    return nc


LIMIT = None  # (max_iters, do_final) for bisection

_NC_CACHE = []


def _get_nc():
    if not _NC_CACHE:
        _NC_CACHE.append(build_nc())
    return _NC_CACHE[0]


def _in_maps(x, Wq, bq, Wk, bk, Wv, bv, Wo, bo):
    bf = lambda a: np.ascontiguousarray(
        np.asarray(a, dtype=np.float32).astype(_BF16))
    f32 = lambda a: np.ascontiguousarray(np.asarray(a, dtype=np.float32))
    shared = {"Wq": bf(Wq), "Wk": bf(Wk), "Wv": bf(Wv), "Wo": bf(Wo),
              "bq": f32(bq), "bk": f32(bk), "bv": f32(bv), "bo": f32(bo)}
    x = np.asarray(x, dtype=np.float32)
    return [{"xT": bf(x[b].T), **shared} for b in range(B)]


def kernel(x, Wq, bq, Wk, bk, Wv, bv, Wo, bo):
    nc = _get_nc()
    in_maps = _in_maps(x, Wq, bq, Wk, bk, Wv, bv, Wo, bo)
    res = run_bass_kernel_spmd(nc, in_maps, list(range(B)))
    return np.stack(
        [np.asarray(res.results[b]["outT"], dtype=np.float32).T
         for b in range(B)], axis=0)


# revision 37
# speedup vs baseline: 1.2881x; 1.0270x over previous
"""DinoV2 attention (B=8, S=1370, D=1024, H=16, Dh=64) on 8 trn2 NeuronCores.

Sharding: data parallel over batch - core b computes batch element b end to
end; weights are replicated; no collectives.

All matmul operands are bf16 (same PE rate as fp32r in the cost model but
half the DMA/SBUF footprint); PSUM accumulation stays fp32.

Structure (per core):
  Phase A: V = x @ Wv + bv (token-major, 512-col chunks) and K^T = Wk^T x
           (feature-major, all keys) back to back, paced by the input DMA
           stream, with all 8 PSUM banks in one rotating ring.
  Main loop, query-chunk-outer (5x256 + 90), head-pair-inner:
    scores S_h = K_h^T Q_h   [keys, qchunk]; two ks tiles x two heads are
    packed into one 2-bank PSUM tile so ONE Act instruction exponentiates
    1024 columns (Act instruction overhead halves).
    O_h = [V_h | 1]^T E_h    [65, qchunk], accumulated over 11 key tiles.
    normalize via reciprocal of row 64 + DRAM-bounce partition broadcast.
    The next iteration's Q-projection passes and the previous query chunk's
    out-projection passes are woven between the attention matmuls so the PE
    never waits on the Act engine's exp stream.
  Out-projection is feature-major (outT = Wo^T @ OT, full 128-deep
  contraction, per-partition bias); the host transposes outT back.
"""

import numpy as np
from contextlib import ExitStack

import concourse.bass as bass
import concourse.mybir as mybir
import concourse.tile as tile
from concourse.bass_utils import run_bass_kernel_spmd

import ml_dtypes
_BF16 = np.dtype(ml_dtypes.bfloat16)

B = 8
S = 1370
D = 1024
H = 16
DH = 64
P = 128
KT = D // P              # 8 contraction tiles over D
NPAIR = H // 2           # 8 head pairs
NST = (S + P - 1) // P   # 11 key tiles (last is 90 rows)
NKP = (NST + 1) // 2     # 6 ks pairs (last is a single)
FP = mybir.dt.float32
BF = mybir.dt.bfloat16
AF = mybir.ActivationFunctionType

ST_SIZES = [min(P, S - i * P) for i in range(NST)]
Q_CHUNKS = [(i * 256, min(256, S - i * 256)) for i in range((S + 255) // 256)]
NQC = len(Q_CHUNKS)      # 6 chunks: 5x256 + 90
K_CHUNKS = [(0, 512), (512, 512), (1024, S - 1024)]
SCALE = 1.0 / np.sqrt(DH)
ONES_BF16_BITS = 0x3F80  # 1.0 in bfloat16


def _legalize_syncs(nc):
    """Move excess sem waits onto injected NoOps.

    This walrus build encodes at most one wait (plus one update) per TPB
    instruction; Tile emits several. Engines execute their streams in
    order and the Tile schedule is a topological order of the dependency
    DAG, so hoisting waits onto preceding same-engine NoOps preserves
    progress (anything scheduled earlier can still complete) and
    correctness (the instruction still starts only after all its waits).
    """
    nid = 0
    for f in nc.m.functions:
        for blk in f.blocks:
            out = []
            for inst in blk.instructions:
                si = inst.sync_info
                if si is not None:
                    waits = list(si.on_wait)
                    ups = list(si.on_update)
                    if len(waits) > 1:
                        for w in waits[:-1]:
                            nop = mybir.InstNoOp(
                                name=f"I-syncfix-{nid}",
                                engine=inst.engine, ins=[], outs=[],
                                sync_info=mybir.SyncInfo(on_wait=[w],
                                                         on_update=[]))
                            nid += 1
                            nc.register_instruction(nop)
                            out.append(nop)
                        inst.sync_info = mybir.SyncInfo(on_wait=waits[-1:],
                                                        on_update=ups)
                out.append(inst)
            blk.instructions = out


def build_nc():
    nc = bass.Bass()
    xT = nc.declare_dram_parameter("xT", [D, S], BF, isOutput=False)
    Wq = nc.declare_dram_parameter("Wq", [D, D], BF, isOutput=False)
    Wk = nc.declare_dram_parameter("Wk", [D, D], BF, isOutput=False)
    Wv = nc.declare_dram_parameter("Wv", [D, D], BF, isOutput=False)
    Wo = nc.declare_dram_parameter("Wo", [D, D], BF, isOutput=False)
    bq = nc.declare_dram_parameter("bq", [D], FP, isOutput=False)
    bk = nc.declare_dram_parameter("bk", [D], FP, isOutput=False)
    bv = nc.declare_dram_parameter("bv", [D], FP, isOutput=False)
    bo = nc.declare_dram_parameter("bo", [D], FP, isOutput=False)
    outT = nc.declare_dram_parameter("outT", [D, S], FP, isOutput=True)
    # bounce buffer for replicating softmax denominators across partitions
    # (SBUF APs can't partition-broadcast, DRAM APs can)
    zdram = nc.dram_tensor("zbounce", [NPAIR, NQC, 2, 256], FP)

    def bcast(handle_slice, nparts):
        a = handle_slice
        return bass.AP(tensor=a.tensor, offset=a.offset,
                       ap=[[0, nparts], *a.ap])

    with ExitStack() as ctx:
        tc = ctx.enter_context(tile.TileContext(nc))
        const = ctx.enter_context(tc.tile_pool(name="const", bufs=1))
        persist = ctx.enter_context(tc.tile_pool(name="persist", bufs=1))
        work = ctx.enter_context(tc.tile_pool(name="work", bufs=1))

        # ---- resident weights / inputs (all bf16) -----------------------
        xT_sb = const.tile([P, KT, S], BF)
        wq_sb = const.tile([P, KT, D], BF)
        wk_sb = const.tile([P, KT, D], BF)
        wv_sb = const.tile([P, KT, D], BF)
        wo_sb = const.tile([P, KT, D], BF)
        # per-partition biases for feature-major projections: feature
        # f = blk*128 + p  ->  [p, blk]
        bq_sb = const.tile([P, KT], FP)
        bk_sb = const.tile([P, KT], FP)
        bo_sb = const.tile([P, KT], FP)
        # V-proj bias is per free-dim column: partition-broadcast via DRAM
        bv_bc = const.tile([P, D], FP)

        xT_r = xT[:].rearrange("(kt p) s -> p kt s", p=P)
        wq_r = Wq[:].rearrange("(kt p) c -> p kt c", p=P)
        wk_r = Wk[:].rearrange("(kt p) c -> p kt c", p=P)
        wv_r = Wv[:].rearrange("(kt p) c -> p kt c", p=P)
        wo_r = Wo[:].rearrange("(kt p) c -> p kt c", p=P)

        # DMA order = arrival order: wv/xT chunk pairs feed phase A first,
        # then wk (phase A's K projection), then the rest.
        nc.sync.dma_start(out=wv_sb[:, 0, 0:512], in_=wv_r[:, 0, 0:512])
        nc.sync.dma_start(out=xT_sb[:, 0, 0:256], in_=xT_r[:, 0, 0:256])
        nc.sync.dma_start(out=wv_sb[:, 0, 512:1024],
                          in_=wv_r[:, 0, 512:1024])
        nc.sync.dma_start(out=xT_sb[:, 0, 256:S], in_=xT_r[:, 0, 256:S])
        nc.sync.dma_start(out=bv_bc, in_=bcast(bv[:], P))
        for kt in range(1, KT):
            nc.sync.dma_start(out=wv_sb[:, kt, :], in_=wv_r[:, kt, :])
            nc.sync.dma_start(out=xT_sb[:, kt, :], in_=xT_r[:, kt, :])
        nc.sync.dma_start(out=bk_sb, in_=bk[:].rearrange("(kt p) -> p kt", p=P))
        nc.sync.dma_start(out=bq_sb, in_=bq[:].rearrange("(kt p) -> p kt", p=P))
        nc.sync.dma_start(out=bo_sb, in_=bo[:].rearrange("(kt p) -> p kt", p=P))
        for kt in range(KT):
            nc.sync.dma_start(out=wk_sb[:, kt, :], in_=wk_r[:, kt, :])
        for kt in range(KT):
            nc.sync.dma_start(out=wq_sb[:, kt, :], in_=wq_r[:, kt, :])
        for kt in range(KT):
            nc.sync.dma_start(out=wo_sb[:, kt, :], in_=wo_r[:, kt, :])

        # ---- persistent activations ------------------------------------
        # V with fused ones column: [keys, st, head, 65]
        v_sb = persist.tile([P, NST, H, DH + 1], BF)
        # K^T feature-major for all head pairs: [feat, hp, key]
        kt_all = persist.tile([P, NPAIR, S], BF)
        # normalized attention output, feature-major: [feat, hp, query]
        ot_sb = persist.tile([P, KT, S], BF)
        nc.vector.memset(
            v_sb[:, :, :, DH:DH + 1].bitcast(mybir.dt.uint16), ONES_BF16_BITS)

        # ---- Phase A: V projection then K^T projection ------------------
        # One 8-bank PSUM ring; V regions are paced by the wv/xT DMA
        # stream, K regions run at full speed once wk has landed.
        with tc.tile_pool(name="psA", bufs=1, space="PSUM") as psA:
            for st in range(NST):
                sw = ST_SIZES[st]
                for half in range(2):
                    c0 = half * 512
                    vps = psA.tile([P, 512], FP, tag="vp", bufs=8)
                    for kt in range(KT):
                        nc.tensor.matmul(
                            vps[:sw, :],
                            lhsT=xT_sb[:, kt, st * P:st * P + sw],
                            rhs=wv_sb[:, kt, c0:c0 + 512],
                            start=(kt == 0), stop=(kt == KT - 1))
                    # evict with bias add; 512 cols = 8 heads x 64 dims
                    nc.vector.tensor_add(
                        v_sb[:sw, st, half * 8:half * 8 + 8, 0:DH],
                        vps[:sw, :].rearrange("p (h d) -> p h d", h=8),
                        bv_bc[:sw, c0:c0 + 512].rearrange(
                            "p (h d) -> p h d", h=8))
            for hp in range(NPAIR):
                for (c0, cwk) in K_CHUNKS:
                    kps = psA.tile([P, 512], FP, tag="vp", bufs=8)
                    for kt in range(KT):
                        nc.tensor.matmul(
                            kps[:, :cwk],
                            lhsT=wk_sb[:, kt, hp * P:(hp + 1) * P],
                            rhs=xT_sb[:, kt, c0:c0 + cwk],
                            start=(kt == 0), stop=(kt == KT - 1))
                    nc.scalar.activation(
                        kt_all[:, hp, c0:c0 + cwk], kps[:, :cwk],
                        AF.Identity, bias=bk_sb[:, hp:hp + 1])

        # ---- main loop: query-chunk-outer, head-pair-inner --------------
        psum = ctx.enter_context(tc.tile_pool(name="psum", bufs=1,
                                              space="PSUM"))
        qt_tiles = {}
        deferred = [None]

        def emit_qproj(hp, qc):
            """Q projection for (hp, query chunk qc)."""
            q0, cw = Q_CHUNKS[qc]
            q_ps = psum.tile([P, 512], FP, tag="qk", bufs=1)
            ops = []
            for kt in range(KT):
                def mm(kt=kt):
                    nc.tensor.matmul(
                        q_ps[:, :cw],
                        lhsT=wq_sb[:, kt, hp * P:(hp + 1) * P],
                        rhs=xT_sb[:, kt, q0:q0 + cw],
                        start=(kt == 0), stop=(kt == KT - 1))
                ops.append(mm)

            def evict():
                qt = work.tile([P, 256], BF, tag="qt", bufs=2)
                nc.vector.tensor_scalar_add(qt[:, :cw], q_ps[:, :cw],
                                            bq_sb[:, hp:hp + 1])
                qt_tiles[(qc, hp)] = qt
            ops.append(evict)
            return ops

        def emit_outproj(qc, pool_tag="op"):
            """Out-projection passes for query chunk qc (all heads done)."""
            q0, cw = Q_CHUNKS[qc]
            ops = []
            op_bank = None
            if pool_tag == "op":
                op_bank = psum.tile([P, 2, 256], FP, tag="op", bufs=1,
                                    name="op_bank")
            for ct in range(KT):
                if pool_tag == "op":
                    # alternate half-bank regions: double-buffering inside
                    # one PSUM bank
                    region = op_bank[:, ct % 2, :cw]
                else:
                    region = None
                for dt in range(KT):
                    def mm(ct=ct, dt=dt, region=region):
                        nc.tensor.matmul(
                            region,
                            lhsT=wo_sb[:, dt, ct * P:(ct + 1) * P],
                            rhs=ot_sb[:, dt, q0:q0 + cw],
                            start=(dt == 0), stop=(dt == KT - 1))
                    ops.append(mm)

                def evict(ct=ct, region=region):
                    o_out = work.tile([P, 256], FP, tag="oout", bufs=2)
                    nc.vector.tensor_scalar_add(o_out[:, :cw], region,
                                                bo_sb[:, ct:ct + 1])
                    nc.sync.dma_start(
                        out=outT[ct * P:(ct + 1) * P, q0:q0 + cw],
                        in_=o_out[:, :cw])
                ops.append(evict)
            return ops

        def emit_outproj_final(qc):
            """Final out-projection in two batches of 4 ct tiles, each ct's
            accumulation group in its own PSUM bank (concurrent groups must
            not share a bank). The o/qk/op banks are idle by now, unlike the
            s ring, which the last attention exps still read. dt-major order
            for dt<7 lets those passes overlap the last head pair's
            normalize chain, whose result only the dt=7 passes need."""
            q0, cw = Q_CHUNKS[qc]
            assert cw <= 256
            for batch in range(2):
                t_o = psum.tile([P, 2, 512], FP, tag="o", bufs=1,
                                name="fin_o")
                t_qk = psum.tile([P, 512], FP, tag="qk", bufs=1,
                                 name="fin_qk")
                t_op = psum.tile([P, 2, 256], FP, tag="op", bufs=1,
                                 name="fin_op")
                cts = range(batch * 4, batch * 4 + 4)
                regions = [t_o[:, 0, :cw], t_o[:, 1, :cw], t_qk[:, :cw],
                           t_op[:, 0, :cw]]
                region = {ct: regions[i] for i, ct in enumerate(cts)}
                for dt in range(KT):
                    for ct in cts:
                        nc.tensor.matmul(
                            region[ct],
                            lhsT=wo_sb[:, dt, ct * P:(ct + 1) * P],
                            rhs=ot_sb[:, dt, q0:q0 + cw],
                            start=(dt == 0), stop=(dt == KT - 1))
                ob = work.tile([P, 4, 256], FP, tag="obat", bufs=2)
                for i, ct in enumerate(cts):
                    nc.vector.tensor_scalar_add(ob[:, i, :cw], region[ct],
                                                bo_sb[:, ct:ct + 1])
                nc.sync.dma_start(
                    out=outT[:].rearrange(
                        "(g p) s -> p g s", p=P)[:, batch * 4:batch * 4 + 4,
                                                 q0:q0 + cw],
                    in_=ob[:, :, :cw])

        def attention(qc, hp, fill):
            """Attention for (qc, hp); pops fill closures between matmuls."""
            q0, cw = Q_CHUNKS[qc]
            qt = qt_tiles.pop((qc, hp))
            # heads in SEPARATE banks: two accumulation groups may not share
            # a PSUM bank's zero region
            o_ps = psum.tile([P, 2, 512], FP, tag="o", bufs=1)

            def scores_exp(kp):
                ks_list = ([2 * kp, 2 * kp + 1] if kp < NKP - 1
                           else [NST - 1])
                # head-major region order: head A tiles live in bank 0,
                # head B tiles in bank 1 - consecutive matmuls with
                # different tile_position row groups must not target the
                # same PSUM bank (device lockup otherwise)
                s_ps = psum.tile([P, 2, 2, 256], FP, tag="s", bufs=2)
                for i, ks in enumerate(ks_list):
                    k0, kw = ks * P, ST_SIZES[ks]
                    nc.tensor.matmul(
                        s_ps[:kw, 0, i, :cw],
                        lhsT=kt_all[0:DH, hp, k0:k0 + kw],
                        rhs=qt[0:DH, :cw], start=True, stop=True)
                    nc.tensor.matmul(
                        s_ps[:kw, 1, i, :cw],
                        lhsT=kt_all[DH:P, hp, k0:k0 + kw],
                        rhs=qt[DH:P, :cw], start=True, stop=True)
                kw0 = ST_SIZES[ks_list[0]]
                e = work.tile([P, 2, 2, 256], BF, tag="e", bufs=5)
                n = len(ks_list)
                nc.scalar.activation(
                    e[:kw0, :, 0:n, :cw],
                    s_ps[:kw0, :, 0:n, :cw],
                    AF.Exp, scale=float(SCALE))
                return (e, ks_list)

            def pv(pend):
                e, ks_list = pend
                for i, ks in enumerate(ks_list):
                    kw = ST_SIZES[ks]
                    for h in range(2):
                        nc.tensor.matmul(
                            o_ps[0:DH + 1, h, :cw],
                            lhsT=v_sb[0:kw, ks, 2 * hp + h, :],
                            rhs=e[:kw, h, i, :cw],
                            start=(ks == 0), stop=(ks == NST - 1))

            # depth-2 software pipeline: pv lags scores by two ks pairs so
            # each exp has ~2 pair-times of latency slack
            pend = [scores_exp(0)]
            # previous iteration's deferred PVs + normalize: running them
            # here gives their exps a full extra pair of latency slack
            if deferred[0] is not None:
                deferred[0]()
                deferred[0] = None
            pend.append(scores_exp(1))
            for kp in range(2, NKP):
                for _ in range(2):
                    if fill:
                        fill.pop(0)()
                pend.append(scores_exp(kp))
                for _ in range(2):
                    if fill:
                        fill.pop(0)()
                pv(pend.pop(0))
            while fill:
                fill.pop(0)()

            def tail(pend=pend, o_ps=o_ps, cw=cw, q0=q0, hp=hp, qc=qc):
                for p in pend:
                    pv(p)
                # copy O out of PSUM immediately so the o banks free for the
                # next iteration (o is single-buffered)
                ocp = work.tile([DH + 1, 2, 256], FP, tag="ocp", bufs=2)
                nc.vector.tensor_copy(ocp[:, :, :cw],
                                      o_ps[0:DH + 1, :, :cw])
                # normalize: row DH holds Z; reciprocal stays on partition
                # 64, then a DRAM bounce replicates it across partitions 0-63
                zt = work.tile([1, 2, 256], FP, tag="zt", bufs=2)
                nc.vector.reciprocal(zt[0:1, :, :cw],
                                     ocp[DH:DH + 1, :, :cw])
                nc.sync.dma_start(out=zdram[hp, qc, :, :cw],
                                  in_=zt[0:1, :, :cw])
                zb = work.tile([DH, 2, 256], FP, tag="zb", bufs=2)
                zsrc = zdram[hp, qc, :, :cw]
                nc.sync.dma_start(out=zb[:, :, :cw], in_=bcast(zsrc, DH))
                # head A lands on partitions 0-63 directly
                nc.vector.tensor_mul(ot_sb[0:DH, hp, q0:q0 + cw],
                                     ocp[0:DH, 0, :cw], zb[:, 0, :cw])
                # head B must land on partitions 64-127: compute at base 0,
                # then DMA-shift partitions
                otb = work.tile([DH, 256], BF, tag="otb", bufs=2)
                nc.gpsimd.tensor_mul(otb[:, :cw], ocp[0:DH, 1, :cw],
                                     zb[:, 1, :cw])
                nc.sync.dma_start(out=ot_sb[DH:P, hp, q0:q0 + cw],
                                  in_=otb[:, :cw])
            deferred[0] = tail

        # lead block: Q projection for the very first iteration
        for op in emit_qproj(0, 0):
            op()

        op_pending = []
        n_it = 0
        for qc in range(NQC):
            for hp in range(NPAIR):
                if LIMIT is not None and n_it >= LIMIT[0]:
                    break
                n_it += 1
                fill = []
                nhp, nqc = hp + 1, qc
                if nhp == NPAIR:
                    nhp, nqc = 0, qc + 1
                if nqc < NQC:
                    fill.extend(emit_qproj(nhp, nqc))
                if qc > 0:
                    if hp == 0:
                        op_pending = emit_outproj(qc - 1)
                    take = 9 if hp < NPAIR - 1 else len(op_pending)
                    fill.extend(op_pending[:take])
                    del op_pending[:take]
                attention(qc, hp, fill)
        if deferred[0] is not None:
            deferred[0]()
            deferred[0] = None
        # final out-projection for the last query chunk
        if LIMIT is None or LIMIT[1]:
            emit_outproj_final(NQC - 1)

    _legalize_syncs(nc)
    return nc


LIMIT = None  # (max_iters, do_final) for bisection

_NC_CACHE = []


def _get_nc():
    if not _NC_CACHE:
        _NC_CACHE.append(build_nc())
    return _NC_CACHE[0]


def _in_maps(x, Wq, bq, Wk, bk, Wv, bv, Wo, bo):
    bf = lambda a: np.ascontiguousarray(
        np.asarray(a, dtype=np.float32).astype(_BF16))
    f32 = lambda a: np.ascontiguousarray(np.asarray(a, dtype=np.float32))
    shared = {"Wq": bf(Wq), "Wk": bf(Wk), "Wv": bf(Wv), "Wo": bf(Wo),
              "bq": f32(bq), "bk": f32(bk), "bv": f32(bv), "bo": f32(bo)}
    x = np.asarray(x, dtype=np.float32)
    return [{"xT": bf(x[b].T), **shared} for b in range(B)]


def kernel(x, Wq, bq, Wk, bk, Wv, bv, Wo, bo):
    nc = _get_nc()
    in_maps = _in_maps(x, Wq, bq, Wk, bk, Wv, bv, Wo, bo)
    res = run_bass_kernel_spmd(nc, in_maps, list(range(B)))
    return np.stack(
        [np.asarray(res.results[b]["outT"], dtype=np.float32).T
         for b in range(B)], axis=0)


# revision 45
# speedup vs baseline: 1.2990x; 1.0084x over previous
"""DinoV2 attention (B=8, S=1370, D=1024, H=16, Dh=64) on 8 trn2 NeuronCores.

Sharding: data parallel over batch - core b computes batch element b end to
end; weights are replicated; no collectives.

All matmul operands are bf16 (same PE rate as fp32r in the cost model but
half the DMA/SBUF footprint); PSUM accumulation stays fp32.

Structure (per core):
  Phase A: V = x @ Wv + bv (token-major, 512-col chunks) and K^T = Wk^T x
           (feature-major, all keys) back to back, paced by the input DMA
           stream, with all 8 PSUM banks in one rotating ring.
  Main loop, query-chunk-outer (5x256 + 90), head-pair-inner:
    scores S_h = K_h^T Q_h   [keys, qchunk]; two ks tiles x two heads are
    packed into one 2-bank PSUM tile so ONE Act instruction exponentiates
    1024 columns (Act instruction overhead halves).
    O_h = [V_h | 1]^T E_h    [65, qchunk], accumulated over 11 key tiles.
    normalize via reciprocal of row 64 + DRAM-bounce partition broadcast.
    The next iteration's Q-projection passes and the previous query chunk's
    out-projection passes are woven between the attention matmuls so the PE
    never waits on the Act engine's exp stream.
  Out-projection is feature-major (outT = Wo^T @ OT, full 128-deep
  contraction, per-partition bias); the host transposes outT back.
"""

import numpy as np
from contextlib import ExitStack

import concourse.bass as bass
import concourse.mybir as mybir
import concourse.tile as tile
from concourse.bass_utils import run_bass_kernel_spmd

import ml_dtypes
_BF16 = np.dtype(ml_dtypes.bfloat16)

B = 8
S = 1370
D = 1024
H = 16
DH = 64
P = 128
KT = D // P              # 8 contraction tiles over D
NPAIR = H // 2           # 8 head pairs
NST = (S + P - 1) // P   # 11 key tiles (last is 90 rows)
NKP = (NST + 1) // 2     # 6 ks pairs (last is a single)
FP = mybir.dt.float32
BF = mybir.dt.bfloat16
AF = mybir.ActivationFunctionType

ST_SIZES = [min(P, S - i * P) for i in range(NST)]
Q_CHUNKS = [(i * 256, min(256, S - i * 256)) for i in range((S + 255) // 256)]
NQC = len(Q_CHUNKS)      # 6 chunks: 5x256 + 90
K_CHUNKS = [(0, 512), (512, 512), (1024, S - 1024)]
SCALE = 1.0 / np.sqrt(DH)
ONES_BF16_BITS = 0x3F80  # 1.0 in bfloat16


def _legalize_syncs(nc):
    """Move excess sem waits onto injected NoOps.

    This walrus build encodes at most one wait (plus one update) per TPB
    instruction; Tile emits several. Engines execute their streams in
    order and the Tile schedule is a topological order of the dependency
    DAG, so hoisting waits onto preceding same-engine NoOps preserves
    progress (anything scheduled earlier can still complete) and
    correctness (the instruction still starts only after all its waits).
    """
    nid = 0
    for f in nc.m.functions:
        for blk in f.blocks:
            out = []
            for inst in blk.instructions:
                si = inst.sync_info
                if si is not None:
                    waits = list(si.on_wait)
                    ups = list(si.on_update)
                    if len(waits) > 1:
                        for w in waits[:-1]:
                            nop = mybir.InstNoOp(
                                name=f"I-syncfix-{nid}",
                                engine=inst.engine, ins=[], outs=[],
                                sync_info=mybir.SyncInfo(on_wait=[w],
                                                         on_update=[]))
                            nid += 1
                            nc.register_instruction(nop)
                            out.append(nop)
                        inst.sync_info = mybir.SyncInfo(on_wait=waits[-1:],
                                                        on_update=ups)
                out.append(inst)
            blk.instructions = out


def build_nc():
    nc = bass.Bass()
    xT = nc.declare_dram_parameter("xT", [D, S], BF, isOutput=False)
    Wq = nc.declare_dram_parameter("Wq", [D, D], BF, isOutput=False)
    Wk = nc.declare_dram_parameter("Wk", [D, D], BF, isOutput=False)
    Wv = nc.declare_dram_parameter("Wv", [D, D], BF, isOutput=False)
    Wo = nc.declare_dram_parameter("Wo", [D, D], BF, isOutput=False)
    bq = nc.declare_dram_parameter("bq", [D], FP, isOutput=False)
    bk = nc.declare_dram_parameter("bk", [D], FP, isOutput=False)
    bv = nc.declare_dram_parameter("bv", [D], FP, isOutput=False)
    bo = nc.declare_dram_parameter("bo", [D], FP, isOutput=False)
    outT = nc.declare_dram_parameter("outT", [D, S], FP, isOutput=True)
    # bounce buffer for replicating softmax denominators across partitions
    # (SBUF APs can't partition-broadcast, DRAM APs can)
    zdram = nc.dram_tensor("zbounce", [NPAIR, NQC, 2, 256], FP)

    def bcast(handle_slice, nparts):
        a = handle_slice
        return bass.AP(tensor=a.tensor, offset=a.offset,
                       ap=[[0, nparts], *a.ap])

    with ExitStack() as ctx:
        tc = ctx.enter_context(tile.TileContext(nc))
        const = ctx.enter_context(tc.tile_pool(name="const", bufs=1))
        persist = ctx.enter_context(tc.tile_pool(name="persist", bufs=1))
        work = ctx.enter_context(tc.tile_pool(name="work", bufs=1))

        # ---- resident weights / inputs (all bf16) -----------------------
        xT_sb = const.tile([P, KT, S], BF)
        wq_sb = const.tile([P, KT, D], BF)
        wk_sb = const.tile([P, KT, D], BF)
        wv_sb = const.tile([P, KT, D], BF)
        wo_sb = const.tile([P, KT, D], BF)
        # per-partition biases for feature-major projections: feature
        # f = blk*128 + p  ->  [p, blk]
        bq_sb = const.tile([P, KT], FP)
        bk_sb = const.tile([P, KT], FP)
        bo_sb = const.tile([P, KT], FP)
        # V-proj bias is per free-dim column: partition-broadcast via DRAM
        bv_bc = const.tile([P, D], FP)

        xT_r = xT[:].rearrange("(kt p) s -> p kt s", p=P)
        wq_r = Wq[:].rearrange("(kt p) c -> p kt c", p=P)
        wk_r = Wk[:].rearrange("(kt p) c -> p kt c", p=P)
        wv_r = Wv[:].rearrange("(kt p) c -> p kt c", p=P)
        wo_r = Wo[:].rearrange("(kt p) c -> p kt c", p=P)

        # DMA order = arrival order: wv/xT chunk pairs feed phase A first,
        # then wk (phase A's K projection), then the rest.
        nc.sync.dma_start(out=wv_sb[:, 0, 0:512], in_=wv_r[:, 0, 0:512])
        nc.sync.dma_start(out=xT_sb[:, 0, 0:256], in_=xT_r[:, 0, 0:256])
        nc.sync.dma_start(out=wv_sb[:, 0, 512:1024],
                          in_=wv_r[:, 0, 512:1024])
        nc.sync.dma_start(out=xT_sb[:, 0, 256:S], in_=xT_r[:, 0, 256:S])
        nc.sync.dma_start(out=bv_bc, in_=bcast(bv[:], P))
        for kt in range(1, KT):
            nc.sync.dma_start(out=wv_sb[:, kt, :], in_=wv_r[:, kt, :])
            nc.sync.dma_start(out=xT_sb[:, kt, :], in_=xT_r[:, kt, :])
        nc.sync.dma_start(out=bk_sb, in_=bk[:].rearrange("(kt p) -> p kt", p=P))
        nc.sync.dma_start(out=bq_sb, in_=bq[:].rearrange("(kt p) -> p kt", p=P))
        nc.sync.dma_start(out=bo_sb, in_=bo[:].rearrange("(kt p) -> p kt", p=P))
        for kt in range(KT):
            nc.sync.dma_start(out=wk_sb[:, kt, :], in_=wk_r[:, kt, :])
        for kt in range(KT):
            nc.sync.dma_start(out=wq_sb[:, kt, :], in_=wq_r[:, kt, :])
        for kt in range(KT):
            nc.sync.dma_start(out=wo_sb[:, kt, :], in_=wo_r[:, kt, :])

        # ---- persistent activations ------------------------------------
        # V with fused ones column: [keys, st, head, 65]
        v_sb = persist.tile([P, NST, H, DH + 1], BF)
        # K^T feature-major for all head pairs: [feat, hp, key]
        kt_all = persist.tile([P, NPAIR, S], BF)
        # normalized attention output, feature-major: [feat, hp, query]
        ot_sb = persist.tile([P, KT, S], BF)
        nc.vector.memset(
            v_sb[:, :, :, DH:DH + 1].bitcast(mybir.dt.uint16), ONES_BF16_BITS)

        # ---- Phase A: V projection then K^T projection ------------------
        # One 8-bank PSUM ring; V regions are paced by the wv/xT DMA
        # stream, K regions run at full speed once wk has landed.
        with tc.tile_pool(name="psA", bufs=1, space="PSUM") as psA:
            for st in range(NST):
                sw = ST_SIZES[st]
                for half in range(2):
                    c0 = half * 512
                    vps = psA.tile([P, 512], FP, tag="vp", bufs=8)
                    for kt in range(KT):
                        nc.tensor.matmul(
                            vps[:sw, :],
                            lhsT=xT_sb[:, kt, st * P:st * P + sw],
                            rhs=wv_sb[:, kt, c0:c0 + 512],
                            start=(kt == 0), stop=(kt == KT - 1))
                    # evict with bias add; 512 cols = 8 heads x 64 dims
                    nc.vector.tensor_add(
                        v_sb[:sw, st, half * 8:half * 8 + 8, 0:DH],
                        vps[:sw, :].rearrange("p (h d) -> p h d", h=8),
                        bv_bc[:sw, c0:c0 + 512].rearrange(
                            "p (h d) -> p h d", h=8))
            for hp in range(NPAIR):
                for (c0, cwk) in K_CHUNKS:
                    kps = psA.tile([P, 512], FP, tag="vp", bufs=8)
                    for kt in range(KT):
                        nc.tensor.matmul(
                            kps[:, :cwk],
                            lhsT=wk_sb[:, kt, hp * P:(hp + 1) * P],
                            rhs=xT_sb[:, kt, c0:c0 + cwk],
                            start=(kt == 0), stop=(kt == KT - 1))
                    nc.scalar.activation(
                        kt_all[:, hp, c0:c0 + cwk], kps[:, :cwk],
                        AF.Identity, bias=bk_sb[:, hp:hp + 1])

        # ---- main loop: query-chunk-outer, head-pair-inner --------------
        psum = ctx.enter_context(tc.tile_pool(name="psum", bufs=1,
                                              space="PSUM"))
        qt_tiles = {}
        deferred = [None]

        def emit_qproj(hp, qc):
            """Q projection for (hp, query chunk qc)."""
            q0, cw = Q_CHUNKS[qc]
            q_ps = psum.tile([P, 512], FP, tag="qk", bufs=1)
            ops = []
            for kt in range(KT):
                def mm(kt=kt):
                    nc.tensor.matmul(
                        q_ps[:, :cw],
                        lhsT=wq_sb[:, kt, hp * P:(hp + 1) * P],
                        rhs=xT_sb[:, kt, q0:q0 + cw],
                        start=(kt == 0), stop=(kt == KT - 1))
                ops.append(mm)

            def evict():
                qt = work.tile([P, 256], BF, tag="qt", bufs=3)
                nc.vector.tensor_scalar_add(qt[:, :cw], q_ps[:, :cw],
                                            bq_sb[:, hp:hp + 1])
                qt_tiles[(qc, hp)] = qt
            ops.append(evict)
            return ops

        def emit_outproj(qc, pool_tag="op"):
            """Out-projection passes for query chunk qc (all heads done)."""
            q0, cw = Q_CHUNKS[qc]
            ops = []
            op_bank = None
            if pool_tag == "op":
                op_bank = psum.tile([P, 2, 256], FP, tag="op", bufs=1,
                                    name="op_bank")
            for ct in range(KT):
                if pool_tag == "op":
                    # alternate half-bank regions: double-buffering inside
                    # one PSUM bank
                    region = op_bank[:, ct % 2, :cw]
                else:
                    region = None
                for dt in range(KT):
                    def mm(ct=ct, dt=dt, region=region):
                        nc.tensor.matmul(
                            region,
                            lhsT=wo_sb[:, dt, ct * P:(ct + 1) * P],
                            rhs=ot_sb[:, dt, q0:q0 + cw],
                            start=(dt == 0), stop=(dt == KT - 1))
                    ops.append(mm)

                def evict(ct=ct, region=region):
                    o_out = work.tile([P, 256], FP, tag="oout", bufs=2)
                    nc.vector.tensor_scalar_add(o_out[:, :cw], region,
                                                bo_sb[:, ct:ct + 1])
                    nc.sync.dma_start(
                        out=outT[ct * P:(ct + 1) * P, q0:q0 + cw],
                        in_=o_out[:, :cw])
                ops.append(evict)
            return ops

        def emit_outproj_final(qc):
            """Final out-projection in two batches of 4 ct tiles, each ct's
            accumulation group in its own PSUM bank (concurrent groups must
            not share a bank). The o/qk/op banks are idle by now, unlike the
            s ring, which the last attention exps still read. dt-major order
            for dt<7 lets those passes overlap the last head pair's
            normalize chain, whose result only the dt=7 passes need."""
            q0, cw = Q_CHUNKS[qc]
            assert cw <= 256
            for batch in range(2):
                t_o = psum.tile([P, 2, 512], FP, tag="o", bufs=1,
                                name="fin_o")
                t_qk = psum.tile([P, 512], FP, tag="qk", bufs=1,
                                 name="fin_qk")
                t_op = psum.tile([P, 2, 256], FP, tag="op", bufs=1,
                                 name="fin_op")
                cts = range(batch * 4, batch * 4 + 4)
                regions = [t_o[:, 0, :cw], t_o[:, 1, :cw], t_qk[:, :cw],
                           t_op[:, 0, :cw]]
                region = {ct: regions[i] for i, ct in enumerate(cts)}
                for dt in range(KT):
                    for ct in cts:
                        nc.tensor.matmul(
                            region[ct],
                            lhsT=wo_sb[:, dt, ct * P:(ct + 1) * P],
                            rhs=ot_sb[:, dt, q0:q0 + cw],
                            start=(dt == 0), stop=(dt == KT - 1))
                ob = work.tile([P, 4, 256], FP, tag="obat", bufs=2)
                for i, ct in enumerate(cts):
                    nc.vector.tensor_scalar_add(ob[:, i, :cw], region[ct],
                                                bo_sb[:, ct:ct + 1])
                nc.sync.dma_start(
                    out=outT[:].rearrange(
                        "(g p) s -> p g s", p=P)[:, batch * 4:batch * 4 + 4,
                                                 q0:q0 + cw],
                    in_=ob[:, :, :cw])

        def attention(qc, hp, fill):
            """Attention for (qc, hp); pops fill closures between matmuls."""
            q0, cw = Q_CHUNKS[qc]
            qt = qt_tiles.pop((qc, hp))
            # heads in SEPARATE banks: two accumulation groups may not share
            # a PSUM bank's zero region
            o_ps = psum.tile([P, 2, 512], FP, tag="o", bufs=1)

            def scores_exp(kp):
                ks_list = ([2 * kp, 2 * kp + 1] if kp < NKP - 1
                           else [NST - 1])
                # head-major region order: head A tiles live in bank 0,
                # head B tiles in bank 1 - consecutive matmuls with
                # different tile_position row groups must not target the
                # same PSUM bank (device lockup otherwise)
                s_ps = psum.tile([P, 2, 2, 256], FP, tag="s", bufs=2)
                for i, ks in enumerate(ks_list):
                    k0, kw = ks * P, ST_SIZES[ks]
                    nc.tensor.matmul(
                        s_ps[:kw, 0, i, :cw],
                        lhsT=kt_all[0:DH, hp, k0:k0 + kw],
                        rhs=qt[0:DH, :cw], start=True, stop=True)
                    nc.tensor.matmul(
                        s_ps[:kw, 1, i, :cw],
                        lhsT=kt_all[DH:P, hp, k0:k0 + kw],
                        rhs=qt[DH:P, :cw], start=True, stop=True)
                kw0 = ST_SIZES[ks_list[0]]
                e = work.tile([P, 2, 2, 256], BF, tag="e", bufs=6)
                n = len(ks_list)
                nc.scalar.activation(
                    e[:kw0, :, 0:n, :cw],
                    s_ps[:kw0, :, 0:n, :cw],
                    AF.Exp, scale=float(SCALE))
                return (e, ks_list)

            def pv(pend):
                e, ks_list = pend
                for i, ks in enumerate(ks_list):
                    kw = ST_SIZES[ks]
                    for h in range(2):
                        nc.tensor.matmul(
                            o_ps[0:DH + 1, h, :cw],
                            lhsT=v_sb[0:kw, ks, 2 * hp + h, :],
                            rhs=e[:kw, h, i, :cw],
                            start=(ks == 0), stop=(ks == NST - 1))

            # depth-2 software pipeline: pv lags scores by two ks pairs so
            # each exp has ~2 pair-times of latency slack
            pend = [scores_exp(0)]
            # previous iteration's deferred PVs + normalize: running them
            # here gives their exps a full extra pair of latency slack
            if deferred[0] is not None:
                deferred[0]()
                deferred[0] = None
            pend.append(scores_exp(1))
            for kp in range(2, NKP):
                for _ in range(3):
                    if fill:
                        fill.pop(0)()
                pend.append(scores_exp(kp))
                if fill:
                    fill.pop(0)()
                pv(pend.pop(0))
            while fill:
                fill.pop(0)()

            def tail(pend=pend, o_ps=o_ps, cw=cw, q0=q0, hp=hp, qc=qc):
                for p in pend:
                    pv(p)
                # copy O out of PSUM immediately so the o banks free for the
                # next iteration (o is single-buffered)
                ocp = work.tile([DH + 1, 2, 256], FP, tag="ocp", bufs=2)
                if qc == NQC - 1 and hp == NPAIR - 1:
                    # final iteration: the Act queue drains first, and the
                    # final out-projection WARs this copy via the o banks
                    nc.scalar.copy(ocp[:, :, :cw], o_ps[0:DH + 1, :, :cw])
                else:
                    nc.vector.tensor_copy(ocp[:, :, :cw],
                                          o_ps[0:DH + 1, :, :cw])
                # normalize: row DH holds Z; reciprocal stays on partition
                # 64, then a DRAM bounce replicates it across partitions 0-63
                zt = work.tile([1, 2, 256], FP, tag="zt", bufs=2)
                nc.vector.reciprocal(zt[0:1, :, :cw],
                                     ocp[DH:DH + 1, :, :cw])
                nc.sync.dma_start(out=zdram[hp, qc, :, :cw],
                                  in_=zt[0:1, :, :cw])
                zb = work.tile([DH, 2, 256], FP, tag="zb", bufs=2)
                zsrc = zdram[hp, qc, :, :cw]
                nc.sync.dma_start(out=zb[:, :, :cw], in_=bcast(zsrc, DH))
                # head A lands on partitions 0-63 directly
                nc.vector.tensor_mul(ot_sb[0:DH, hp, q0:q0 + cw],
                                     ocp[0:DH, 0, :cw], zb[:, 0, :cw])
                # head B must land on partitions 64-127: compute at base 0,
                # then DMA-shift partitions
                otb = work.tile([DH, 256], BF, tag="otb", bufs=2)
                nc.gpsimd.tensor_mul(otb[:, :cw], ocp[0:DH, 1, :cw],
                                     zb[:, 1, :cw])
                nc.sync.dma_start(out=ot_sb[DH:P, hp, q0:q0 + cw],
                                  in_=otb[:, :cw])
            deferred[0] = tail

        # lead block: Q projection for the very first iteration
        for op in emit_qproj(0, 0):
            op()

        op_pending = []
        n_it = 0
        for qc in range(NQC):
            for hp in range(NPAIR):
                if LIMIT is not None and n_it >= LIMIT[0]:
                    break
                n_it += 1
                fill = []
                nhp, nqc = hp + 1, qc
                if nhp == NPAIR:
                    nhp, nqc = 0, qc + 1
                if nqc < NQC:
                    fill.extend(emit_qproj(nhp, nqc))
                if qc > 0:
                    if hp == 0:
                        op_pending = emit_outproj(qc - 1)
                    # back-loaded: later iterations get more fill, letting
                    # the Act queue drain before the end of the chunk
                    take = (6, 6, 7, 8, 9, 10, 12)[hp] \
                        if hp < NPAIR - 1 else len(op_pending)
                    fill.extend(op_pending[:take])
                    del op_pending[:take]
                attention(qc, hp, fill)
        if deferred[0] is not None:
            deferred[0]()
            deferred[0] = None
        # final out-projection for the last query chunk
        if LIMIT is None or LIMIT[1]:
            emit_outproj_final(NQC - 1)

    _legalize_syncs(nc)
    return nc


LIMIT = None  # (max_iters, do_final) for bisection

_NC_CACHE = []


def _get_nc():
    if not _NC_CACHE:
        _NC_CACHE.append(build_nc())
    return _NC_CACHE[0]


def _in_maps(x, Wq, bq, Wk, bk, Wv, bv, Wo, bo):
    bf = lambda a: np.ascontiguousarray(
        np.asarray(a, dtype=np.float32).astype(_BF16))
    f32 = lambda a: np.ascontiguousarray(np.asarray(a, dtype=np.float32))
    shared = {"Wq": bf(Wq), "Wk": bf(Wk), "Wv": bf(Wv), "Wo": bf(Wo),
              "bq": f32(bq), "bk": f32(bk), "bv": f32(bv), "bo": f32(bo)}
    x = np.asarray(x, dtype=np.float32)
    return [{"xT": bf(x[b].T), **shared} for b in range(B)]


def kernel(x, Wq, bq, Wk, bk, Wv, bv, Wo, bo):
    nc = _get_nc()
    in_maps = _in_maps(x, Wq, bq, Wk, bk, Wv, bv, Wo, bo)
    res = run_bass_kernel_spmd(nc, in_maps, list(range(B)))
    return np.stack(
        [np.asarray(res.results[b]["outT"], dtype=np.float32).T
         for b in range(B)], axis=0)
